# revision 8
# baseline (speedup 1.0000x reference)
"""Trainium2 Bass kernel for nn_MultiHeadAttention (B=4, S=2048, D=512, H=8).

Sharding: tensor-parallel over heads — core c owns head c (Dh=64).
Each core computes q/k/v projections for its head slice (full x replicated),
attention for its head over all 4 batches, and the partial out-projection
O_c @ Wo[c]; the host sums the 8 partials (the TP all-reduce done at gather
time) and adds the biases that commute with that reduction (bo, bv@Wo).

Per-core pipeline (all matmul operands float32r = full-rate fp32):
  1. x[b] loads naturally; x^T chunks built with PE identity-transposes.
  2. Q^T, K^T [64, S] via W-stationary matmuls (+ bq/bk at PSUM evac);
     batches are PAIRED: [Q^T_even; Q^T_odd] stacked on partitions 0-63 /
     64-127 (odd half placed via SBUF->SBUF DMA partition shift), so the
     dh=64-contraction S^T matmuls can run ROW-TILED (64x128 PE mode, two
     batches on independent array halves — a K=64 matmul alone leaves half
     the array idle and HAM then pins the PE clock to 1.2 GHz).
  3. V^T via W-stationary, PE-transposed into V_aug [128k, 66] tiles with a
     ones column (softmax denominators land in O^T_aug row 64).
  4. S^T[k,q] = K^T.T @ Q^T row-tiled pairs; exp(S/8) on ACT (no max
     subtraction: |logits| < ~3 by construction).
  5. O^T_aug[66, q] += V_aug.T @ P^T (full-K matmuls).
  6. Out-proj: lhsT = O^T_aug chunk, rhs = [Wo_c; 0] plus an e-column that
     lands the denominator on the token partition; DVE divides during evac.
"""
import numpy as np

import concourse.bass as bass
import concourse.mybir as mybir
import concourse.tile as tile
from concourse import bacc
from concourse.bass_utils import run_bass_kernel_spmd

B, S, D = 4, 2048, 512
H, DH = 8, 64
NCORES = 8
F32 = mybir.dt.float32
F32R = mybir.dt.float32r
AF = mybir.ActivationFunctionType

_NC_CACHE = {}


def build_kernel():
    nc = bacc.Bacc("TRN2", target_bir_lowering=False, debug=False)

    x = nc.dram_tensor("x", [B, S, D], F32R, kind="ExternalInput")
    wq = nc.dram_tensor("wq", [D, DH], F32R, kind="ExternalInput")
    wk = nc.dram_tensor("wk", [D, DH], F32R, kind="ExternalInput")
    wv = nc.dram_tensor("wv", [D, DH], F32R, kind="ExternalInput")
    wo_aug = nc.dram_tensor("wo_aug", [DH + 2, D + 2], F32R, kind="ExternalInput")
    bq = nc.dram_tensor("bq", [DH, 1], F32, kind="ExternalInput")
    bk = nc.dram_tensor("bk", [DH, 1], F32, kind="ExternalInput")
    idin = nc.dram_tensor("idin", [128, 128], F32R, kind="ExternalInput")
    onesin = nc.dram_tensor("onesin", [128, 16, 2], F32R, kind="ExternalInput")
    out = nc.dram_tensor("out", [B * S, D], F32, kind="ExternalOutput")

    NKT = S // 128          # 16 k/token tiles per batch
    NQB = S // 512          # 4 512-blocks per batch
    NCH = D // 128          # 4 dm chunks

    with tile.TileContext(nc) as tc:
        with (
            tc.tile_pool(name="consts", bufs=1) as consts,
            tc.tile_pool(name="xnp", bufs=6) as xnp,
            tc.tile_pool(name="xtp", bufs=6) as xtp,
            tc.tile_pool(name="qkp", bufs=2) as qkp,
            tc.tile_pool(name="stgp", bufs=4) as stgp,
            tc.tile_pool(name="vtp", bufs=2) as vtp,
            tc.tile_pool(name="vp", bufs=2) as vp,
            tc.tile_pool(name="ptp", bufs=3) as ptp,
            tc.tile_pool(name="otp", bufs=2) as otp,
            tc.tile_pool(name="outp", bufs=4) as outp,
            tc.tile_pool(name="rcp", bufs=4) as rcp,
            tc.tile_pool(name="psA", bufs=2, space="PSUM") as psA,     # [128,1024] x2 = 4 banks
            tc.tile_pool(name="psO", bufs=2, space="PSUM") as psO,     # [66,1024] x2 = 4 banks (po + misc)
        ):
            # --- constants ---
            wq_sb = consts.tile([128, NCH, DH], F32R)
            wk_sb = consts.tile([128, NCH, DH], F32R)
            wv_sb = consts.tile([128, NCH, DH], F32R)
            wo_sb = consts.tile([DH + 2, D + 2], F32R)
            bq_sb = consts.tile([DH, 1], F32)
            bk_sb = consts.tile([DH, 1], F32)
            ident = consts.tile([128, 128], F32R)
            nc.sync.dma_start(out=wq_sb[:], in_=wq.rearrange("(c p) m -> p c m", p=128))
            nc.sync.dma_start(out=wk_sb[:], in_=wk.rearrange("(c p) m -> p c m", p=128))
            nc.sync.dma_start(out=wv_sb[:], in_=wv.rearrange("(c p) m -> p c m", p=128))
            nc.sync.dma_start(out=wo_sb[:], in_=wo_aug[:])
            nc.sync.dma_start(out=bq_sb[:], in_=bq[:])
            nc.sync.dma_start(out=bk_sb[:], in_=bk[:])
            nc.sync.dma_start(out=ident[:], in_=idin[:])

            for pr in range(B // 2):
                qt_p = qkp.tile([128, S], F32R, tag="qt", name=f"qt_{pr}")
                kt_p = qkp.tile([128, S], F32R, tag="kt", name=f"kt_{pr}")
                v_bs, ot_bs = [], []
                for half in range(2):
                    b = pr * 2 + half
                    # --- load x naturally; build x^T chunks via PE transpose ---
                    xt = []
                    for ci in range(NCH):
                        xt_c = xtp.tile([128, S], F32R, tag="xt", name=f"xt_{b}_{ci}")
                        xt.append(xt_c)
                    for blk in range(NQB):
                        xn4 = []
                        for j in range(4):
                            xn_t = xnp.tile([128, D], F32R, tag="xn", name=f"xn_{b}_{blk}_{j}")
                            nc.sync.dma_start(
                                out=xn_t[:], in_=x[b, bass.ds(blk * 512 + j * 128, 128), :]
                            )
                            xn4.append(xn_t)
                        for ci in range(NCH):
                            pxt = psA.tile([128, 512], F32R, tag="psA", name=f"pxt_{b}_{blk}_{ci}")
                            for j in range(4):
                                nc.tensor.transpose(
                                    pxt[:, bass.ts(j, 128)],
                                    xn4[j][:, bass.ts(ci, 128)],
                                    ident[:],
                                )
                            nc.vector.tensor_copy(xt[ci][:, bass.ts(blk, 512)], pxt[:])

                    # --- Q^T, K^T projections (W-stationary) into pair rows ---
                    for blk in range(NQB):
                        sl = bass.ts(blk, 512)
                        pq = psO.tile([DH, 512], F32, tag="psO", name=f"pq_{b}_{blk}")
                        pk = psO.tile([DH, 512], F32, tag="psO", name=f"pk_{b}_{blk}")
                        for ci in range(NCH):
                            nc.tensor.matmul(
                                pq[:], wq_sb[:, ci, :], xt[ci][:, sl],
                                start=(ci == 0), stop=(ci == NCH - 1),
                            )
                        for ci in range(NCH):
                            nc.tensor.matmul(
                                pk[:], wk_sb[:, ci, :], xt[ci][:, sl],
                                start=(ci == 0), stop=(ci == NCH - 1),
                            )
                        if half == 0:
                            nc.scalar.activation(qt_p[0:DH, sl], pq[:], AF.Identity, bias=bq_sb[:])
                            nc.scalar.activation(kt_p[0:DH, sl], pk[:], AF.Identity, bias=bk_sb[:])
                        else:
                            sq = stgp.tile([DH, 512], F32R, tag="stg", name=f"sq_{b}_{blk}")
                            sk = stgp.tile([DH, 512], F32R, tag="stg", name=f"sk_{b}_{blk}")
                            nc.scalar.activation(sq[:], pq[:], AF.Identity, bias=bq_sb[:])
                            nc.scalar.activation(sk[:], pk[:], AF.Identity, bias=bk_sb[:])
                            nc.sync.dma_start(out=qt_p[DH:128, sl], in_=sq[:])
                            nc.sync.dma_start(out=kt_p[DH:128, sl], in_=sk[:])

                    # --- V^T projection, then PE-transpose into V_aug ---
                    vt_b = vtp.tile([DH, S], F32R, tag="vt", name=f"vt_{b}")
                    for blk in range(NQB):
                        sl = bass.ts(blk, 512)
                        pvt = psO.tile([DH, 512], F32, tag="psO", name=f"pvt_{b}_{blk}")
                        for ci in range(NCH):
                            nc.tensor.matmul(
                                pvt[:], wv_sb[:, ci, :], xt[ci][:, sl],
                                start=(ci == 0), stop=(ci == NCH - 1),
                            )
                        nc.scalar.copy(vt_b[:, sl], pvt[:])
                    v_b = vp.tile([128, NKT, DH + 2], F32R, tag="v", name=f"v_{b}")
                    nc.sync.dma_start(out=v_b[:, :, DH:DH + 2], in_=onesin[:])
                    for vh in range(2):
                        pvtr = psA.tile([128, 512], F32R, tag="psA", name=f"pvtr_{b}_{vh}")
                        for j in range(8):
                            nc.tensor.transpose(
                                pvtr[:, bass.ts(j, 64)],
                                vt_b[:, bass.ts(vh * 8 + j, 128)],
                                ident[0:DH, 0:DH],
                            )
                        nc.vector.tensor_copy(
                            v_b[:, bass.ds(vh * 8, 8), 0:DH],
                            pvtr[:].rearrange("p (k m) -> p k m", m=64),
                        )
                    v_bs.append(v_b)
                    ot_bs.append(otp.tile([DH + 2, S], F32R, tag="ot", name=f"ot_{b}"))

                # --- attention (row-tiled S^T pairs) ---
                for qh in range(2):  # 1024-wide q halves
                    sl_q = bass.ds(qh * 1024, 1024)
                    po0 = psO.tile([DH + 2, 1024], F32, tag="psO", name=f"po0_{pr}_{qh}")
                    po1 = psO.tile([DH + 2, 1024], F32, tag="psO", name=f"po1_{pr}_{qh}")
                    po = [po0, po1]
                    for kt_i in range(NKT):
                        kt_sl = bass.ts(kt_i, 128)
                        ptts = []
                        for hb in range(2):
                            prt = (hb * DH, 0)
                            pst = psA.tile([128, 1024], F32, tag="psA", name=f"pst_{pr}_{qh}_{kt_i}_{hb}")
                            for j in range(2):
                                nc.tensor.matmul(
                                    pst[:, bass.ts(j, 512)],
                                    kt_p[hb * DH:(hb + 1) * DH, kt_sl],
                                    qt_p[hb * DH:(hb + 1) * DH, bass.ds(qh * 1024 + j * 512, 512)],
                                    start=True, stop=True,
                                    tile_position=prt,
                                )
                            ptt = ptp.tile([128, 1024], F32R, tag="pt", name=f"ptt_{pr}_{qh}_{kt_i}_{hb}")
                            nc.scalar.activation(ptt[:], pst[:], AF.Exp, scale=0.125)
                            ptts.append(ptt)
                        for hb in range(2):
                            for j in range(2):
                                nc.tensor.matmul(
                                    po[hb][:, bass.ts(j, 512)],
                                    v_bs[hb][:, kt_i, :],
                                    ptts[hb][:, bass.ts(j, 512)],
                                    start=(kt_i == 0), stop=(kt_i == NKT - 1),
                                )
                    for hb in range(2):
                        nc.vector.tensor_copy(ot_bs[hb][:, sl_q], po[hb][:])

                # --- out-projection + normalize ---
                for half in range(2):
                    b = pr * 2 + half
                    ot_b = ot_bs[half]
                    for tt in range(NKT):
                        pop = psA.tile([128, 512], F32, tag="psA", name=f"pop_{b}_{tt}")
                        pos = psO.tile([128, 2], F32, tag="psO", name=f"pos_{b}_{tt}")
                        otc = ot_b[:, bass.ts(tt, 128)]
                        nc.tensor.matmul(pop[:], otc, wo_sb[:, 0:D], start=True, stop=True)
                        nc.tensor.matmul(pos[:], otc, wo_sb[:, D:D + 2], start=True, stop=True)
                        rc = rcp.tile([128, 1], F32, tag="rc", name=f"rc_{b}_{tt}")
                        nc.vector.reciprocal(rc[:], pos[:, 0:1])
                        so = outp.tile([128, 512], F32, tag="so", name=f"so_{b}_{tt}")
                        nc.vector.tensor_scalar_mul(so[:], pop[:], rc[:])
                        nc.sync.dma_start(
                            out=out[bass.ds(b * S + tt * 128, 128), :], in_=so[:]
                        )

    nc.compile()
    return nc


def kernel(x, Wq, bq, Wk, bk, Wv, bv, Wo, bo):
    x = np.ascontiguousarray(np.asarray(x, dtype=np.float32))
    Wq = np.asarray(Wq, dtype=np.float32)
    Wk = np.asarray(Wk, dtype=np.float32)
    Wv = np.asarray(Wv, dtype=np.float32)
    Wo = np.asarray(Wo, dtype=np.float32)
    bq = np.asarray(bq, dtype=np.float32)
    bk = np.asarray(bk, dtype=np.float32)
    bv = np.asarray(bv, dtype=np.float32)
    bo = np.asarray(bo, dtype=np.float32)

    if "nc" not in _NC_CACHE:
        _NC_CACHE["nc"] = build_kernel()
    nc = _NC_CACHE["nc"]

    eye = np.eye(128, dtype=np.float32)
    ones = np.zeros((128, 16, 2), dtype=np.float32)
    ones[:, :, 0] = 1.0
    in_maps = []
    for c in range(NCORES):
        hs = slice(c * DH, (c + 1) * DH)
        wo_a = np.zeros((DH + 2, D + 2), dtype=np.float32)
        wo_a[0:DH, 0:D] = Wo[hs, :]
        wo_a[DH, D] = 1.0
        in_maps.append({
            "x": x,
            "wq": np.ascontiguousarray(Wq[:, hs]),
            "wk": np.ascontiguousarray(Wk[:, hs]),
            "wv": np.ascontiguousarray(Wv[:, hs]),
            "wo_aug": wo_a,
            "bq": np.ascontiguousarray(bq[hs].reshape(DH, 1)),
            "bk": np.ascontiguousarray(bk[hs].reshape(DH, 1)),
            "idin": eye,
            "onesin": ones,
        })

    res = run_bass_kernel_spmd(nc, in_maps, list(range(NCORES)))

    acc = np.zeros((B * S, D), dtype=np.float32)
    for c in range(NCORES):
        acc += res.results[c]["out"]
    # biases that commute with the head-reduction, applied at gather time
    acc += bo[None, :] + (bv @ Wo)[None, :]
    return acc.reshape(B, S, D)


# revision 22
# speedup vs baseline: 2.0199x; 2.0199x over previous
"""Trainium2 Bass kernel for nn_MultiHeadAttention (B=4, S=2048, D=512, H=8).

Sharding: tensor-parallel over heads — core c owns head c (Dh=64).
Each core computes q/k/v projections for its head slice (full x replicated,
host-pre-transposed to x^T in bf16), attention for its head over all 4
batches, and the partial out-projection O_c @ Wo[c]; the host sums the 8
partials (the TP all-reduce done at gather time) and adds the biases that
commute with that reduction (bo, bv@Wo).

Engine plan (engines execute their programs in order, so emission order IS
the software pipeline):
  - PE: projections (W-stationary bf16), V^T->V PE-transposes, row-tiled
    S^T pairs (dh=64 contraction on 64x128 array halves), AV with a ones
    column (softmax denominators ride in O^T_aug row 64), out-projection.
  - ACT: exclusively exp(S/8) — it is the pacing engine (~143us floor);
    anything else queued on ACT delays the next attention phase.
  - DVE: all PSUM evacuations + the final normalize (reciprocal of the
    denominator extracted per token partition by an e-column matmul).
  - GPSIMD: bulk x^T loads;  SP: staging shifts + output stores.
Batches are paired [even; odd] on SBUF partition halves; prep of pair 1 is
emitted interleaved with attention of pair 0, and both out-projections are
emitted interleaved with attention of pair 1.
"""
import numpy as np

import concourse.bass as bass
import concourse.mybir as mybir
import concourse.tile as tile
from concourse import bacc
from concourse.bass_utils import run_bass_kernel_spmd

B, S, D = 4, 2048, 512
H, DH = 8, 64
NCORES = 8
F32 = mybir.dt.float32
F32R = mybir.dt.float32r
BF16 = mybir.dt.bfloat16
AF = mybir.ActivationFunctionType

NKT = S // 128          # 16 token tiles per batch
NQB = S // 512          # 4 512-blocks per batch
NCH = D // 128          # 4 dm chunks

_NC_CACHE = {}


def build_kernel():
    nc = bacc.Bacc("TRN2", target_bir_lowering=False, debug=False)

    xT = nc.dram_tensor("xT", [B, D, S], BF16, kind="ExternalInput")
    wq = nc.dram_tensor("wq", [D, DH], BF16, kind="ExternalInput")
    wk = nc.dram_tensor("wk", [D, DH], BF16, kind="ExternalInput")
    wv = nc.dram_tensor("wv", [D, DH], BF16, kind="ExternalInput")
    wo_aug = nc.dram_tensor("wo_aug", [DH + 2, D + 2], BF16, kind="ExternalInput")
    bq = nc.dram_tensor("bq", [DH, 1], F32, kind="ExternalInput")
    bk = nc.dram_tensor("bk", [DH, 1], F32, kind="ExternalInput")
    idin = nc.dram_tensor("idin", [128, 128], BF16, kind="ExternalInput")
    onesin = nc.dram_tensor("onesin", [128, 16, 2], BF16, kind="ExternalInput")
    out = nc.dram_tensor("out", [B * S, D], F32, kind="ExternalOutput")

    with tile.TileContext(nc) as tc:
        with (
            tc.tile_pool(name="consts", bufs=1) as consts,
            tc.tile_pool(name="xtp", bufs=16) as xtp,
            tc.tile_pool(name="qkp", bufs=2) as qkp,
            tc.tile_pool(name="stgp", bufs=4) as stgp,
            tc.tile_pool(name="vtp", bufs=4) as vtp,
            tc.tile_pool(name="vp", bufs=4) as vp,
            tc.tile_pool(name="ptp", bufs=3) as ptp,
            tc.tile_pool(name="otp", bufs=4) as otp,
            tc.tile_pool(name="outp", bufs=6) as outp,
            tc.tile_pool(name="rcp", bufs=6) as rcp,
            tc.tile_pool(name="psA", bufs=2, space="PSUM") as psA,   # pst [128,1024] x2
            tc.tile_pool(name="psO", bufs=2, space="PSUM") as psO,   # po [66,512] x2
            tc.tile_pool(name="psM", bufs=2, space="PSUM") as psM,   # misc [128,512] x2
        ):
            wq_sb = consts.tile([128, NCH, DH], BF16)
            wk_sb = consts.tile([128, NCH, DH], BF16)
            wv_sb = consts.tile([128, NCH, DH], BF16)
            wo_sb = consts.tile([DH + 2, D + 2], BF16)
            bq_sb = consts.tile([DH, 1], F32)
            bk_sb = consts.tile([DH, 1], F32)
            ident = consts.tile([128, 128], BF16)
            nc.sync.dma_start(out=wq_sb[:], in_=wq.rearrange("(c p) m -> p c m", p=128))
            nc.sync.dma_start(out=wk_sb[:], in_=wk.rearrange("(c p) m -> p c m", p=128))
            nc.sync.dma_start(out=wv_sb[:], in_=wv.rearrange("(c p) m -> p c m", p=128))
            nc.sync.dma_start(out=wo_sb[:], in_=wo_aug[:])
            nc.sync.dma_start(out=bq_sb[:], in_=bq[:])
            nc.sync.dma_start(out=bk_sb[:], in_=bk[:])
            nc.sync.dma_start(out=ident[:], in_=idin[:])

            state = {}

            def alloc_pair(pr):
                st = {"xt": {}, "vt": {}, "v": {}, "ot": {}}
                st["qt"] = qkp.tile([128, S], BF16, tag="qt", name=f"qt_{pr}")
                st["kt"] = qkp.tile([128, S], BF16, tag="kt", name=f"kt_{pr}")
                for half in range(2):
                    b = pr * 2 + half
                    st["vt"][half] = vtp.tile([DH, S], BF16, tag="vt", name=f"vt_{b}")
                state[pr] = st

            def emit_xt_loads(pr):
                st = state[pr]
                for half in range(2):
                    b = pr * 2 + half
                    xts = []
                    for ci in range(NCH):
                        xt_c = xtp.tile([128, S], BF16, tag="xt", name=f"xt_{b}_{ci}")
                        xts.append(xt_c)
                    for blk in range(NQB):
                        for ci in range(NCH):
                            nc.gpsimd.dma_start(
                                out=xts[ci][:, bass.ts(blk, 512)],
                                in_=xT[b, bass.ts(ci, 128), bass.ts(blk, 512)],
                            )
                    st["xt"][half] = xts

            def emit_prep_blk(pr, blk):
                """QK + V^T projections for both pair halves, one 512-token
                block; V transposes ride after blocks 1 and 3."""
                st = state[pr]
                sl = bass.ts(blk, 512)
                for half in range(2):
                    b = pr * 2 + half
                    xt = st["xt"][half]
                    pqk = psM.tile([128, 512], F32, tag="psM", name=f"pqk_{b}_{blk}")
                    pq = pqk[0:DH, :]
                    pk = pqk[DH:128, :]
                    for ci in range(NCH):
                        nc.tensor.matmul(
                            pq, wq_sb[:, ci, :], xt[ci][:, sl],
                            start=(ci == 0), stop=(ci == NCH - 1),
                        )
                    for ci in range(NCH):
                        nc.tensor.matmul(
                            pk, wk_sb[:, ci, :], xt[ci][:, sl],
                            start=(ci == 0), stop=(ci == NCH - 1),
                        )
                    if half == 0:
                        nc.vector.tensor_scalar_add(st["qt"][0:DH, sl], pq, bq_sb[:])
                        nc.vector.tensor_scalar_add(st["kt"][0:DH, sl], pk, bk_sb[:])
                    else:
                        sq = stgp.tile([DH, 512], BF16, tag="stg", name=f"sq_{b}_{blk}")
                        sk = stgp.tile([DH, 512], BF16, tag="stg", name=f"sk_{b}_{blk}")
                        nc.vector.tensor_scalar_add(sq[:], pq, bq_sb[:])
                        nc.vector.tensor_scalar_add(sk[:], pk, bk_sb[:])
                        nc.sync.dma_start(out=st["qt"][DH:128, sl], in_=sq[:])
                        nc.sync.dma_start(out=st["kt"][DH:128, sl], in_=sk[:])
                    pvt = psM.tile([DH, 512], F32, tag="psM", name=f"pvt_{b}_{blk}")
                    for ci in range(NCH):
                        nc.tensor.matmul(
                            pvt[:], wv_sb[:, ci, :], xt[ci][:, sl],
                            start=(ci == 0), stop=(ci == NCH - 1),
                        )
                    nc.vector.tensor_copy(st["vt"][half][:, sl], pvt[:])
                if blk in (1, 3):
                    vh = blk // 2
                    for half in range(2):
                        b = pr * 2 + half
                        if vh == 0:
                            v_b = vp.tile([128, NKT, DH + 2], BF16, tag="v", name=f"v_{b}")
                            nc.gpsimd.dma_start(out=v_b[:, :, DH:DH + 2], in_=onesin[:])
                            st["v"][half] = v_b
                        v_b = st["v"][half]
                        pvtr = psM.tile([128, 512], BF16, tag="psM", name=f"pvtr_{b}_{vh}")
                        for j in range(8):
                            nc.tensor.transpose(
                                pvtr[:, bass.ts(j, 64)],
                                st["vt"][half][:, bass.ts(vh * 8 + j, 128)],
                                ident[0:DH, 0:DH],
                            )
                        nc.vector.tensor_copy(
                            v_b[:, bass.ds(vh * 8, 8), 0:DH],
                            pvtr[:].rearrange("p (k m) -> p k m", m=64),
                        )

            def emit_attn_qq(pr, qq):
                st = state[pr]
                with nc.named_scope(f"attn_{pr}_{qq}"):
                    sl_q = bass.ts(qq, 512)
                    if qq == 0:
                        for half in range(2):
                            st["ot"][half] = otp.tile(
                                [DH + 2, S], BF16, tag="ot", name=f"ot_{pr * 2 + half}"
                            )
                    po = [
                        psO.tile([DH + 2, 512], F32, tag="psO", name=f"po{hb}_{pr}_{qq}")
                        for hb in range(2)
                    ]
                    for kt_i in range(NKT):
                        kt_sl = bass.ts(kt_i, 128)
                        pst = psA.tile([128, 1024], F32, tag="psA", name=f"pst_{pr}_{qq}_{kt_i}")
                        for hb in range(2):
                            nc.tensor.matmul(
                                pst[:, bass.ts(hb, 512)],
                                st["kt"][hb * DH:(hb + 1) * DH, kt_sl],
                                st["qt"][hb * DH:(hb + 1) * DH, sl_q],
                                start=True, stop=True,
                                tile_position=(hb * DH, 0),
                            )
                        ptt = ptp.tile([128, 1024], BF16, tag="pt", name=f"ptt_{pr}_{qq}_{kt_i}")
                        nc.scalar.activation(ptt[:], pst[:], AF.Exp, scale=0.125)
                        for hb in range(2):
                            nc.tensor.matmul(
                                po[hb][:],
                                st["v"][hb][:, kt_i, :],
                                ptt[:, bass.ts(hb, 512)],
                                start=(kt_i == 0), stop=(kt_i == NKT - 1),
                            )
                    for hb in range(2):
                        nc.vector.tensor_copy(st["ot"][hb][:, sl_q], po[hb][:])

            def emit_op_tts(pr, half, tts):
                st = state[pr]
                b = pr * 2 + half
                ot_b = st["ot"][half]
                for tt in tts:
                    pop = psM.tile([128, 512], F32, tag="psM", name=f"pop_{b}_{tt}")
                    pos = psM.tile([128, 2], F32, tag="psM", name=f"pos_{b}_{tt}")
                    otc = ot_b[:, bass.ts(tt, 128)]
                    nc.tensor.matmul(pop[:], otc, wo_sb[:, 0:D], start=True, stop=True)
                    nc.tensor.matmul(pos[:], otc, wo_sb[:, D:D + 2], start=True, stop=True)
                    rc = rcp.tile([128, 1], F32, tag="rc", name=f"rc_{b}_{tt}")
                    nc.vector.reciprocal(rc[:], pos[:, 0:1])
                    so = outp.tile([128, 512], F32, tag="so", name=f"so_{b}_{tt}")
                    nc.vector.tensor_scalar_mul(so[:], pop[:], rc[:])
                    nc.sync.dma_start(
                        out=out[bass.ds(b * S + tt * 128, 128), :], in_=so[:]
                    )

            # ---------------- emission schedule ----------------
            alloc_pair(0)
            alloc_pair(1)
            emit_xt_loads(0)
            emit_xt_loads(1)
            for blk in range(NQB):
                emit_prep_blk(0, blk)
            for qq in range(NQB):
                emit_attn_qq(0, qq)
                emit_prep_blk(1, qq)       # pair-1 prep rides under pair-0 attention
            for qq in range(NQB):
                emit_attn_qq(1, qq)
                # pair-0 out-proj rides under pair-1 attention
                emit_op_tts(0, 0, range(qq * 4, qq * 4 + 4))
                emit_op_tts(0, 1, range(qq * 4, qq * 4 + 4))
                # pair-1 out-proj for already-evacuated q blocks
                if qq > 0:
                    emit_op_tts(1, 0, range((qq - 1) * 4, qq * 4))
                    emit_op_tts(1, 1, range((qq - 1) * 4, qq * 4))
            emit_op_tts(1, 0, range(12, 16))
            emit_op_tts(1, 1, range(12, 16))

    nc.compile()
    return nc


def kernel(x, Wq, bq, Wk, bk, Wv, bv, Wo, bo):
    import ml_dtypes
    x = np.asarray(x, dtype=np.float32)
    xT = np.ascontiguousarray(np.transpose(x, (0, 2, 1))).astype(ml_dtypes.bfloat16)
    Wq = np.asarray(Wq, dtype=np.float32)
    Wk = np.asarray(Wk, dtype=np.float32)
    Wv = np.asarray(Wv, dtype=np.float32)
    Wo = np.asarray(Wo, dtype=np.float32)
    bq = np.asarray(bq, dtype=np.float32)
    bk = np.asarray(bk, dtype=np.float32)
    bv = np.asarray(bv, dtype=np.float32)
    bo = np.asarray(bo, dtype=np.float32)

    if "nc" not in _NC_CACHE:
        _NC_CACHE["nc"] = build_kernel()
    nc = _NC_CACHE["nc"]

    eye = np.eye(128).astype(ml_dtypes.bfloat16)
    ones = np.zeros((128, 16, 2), dtype=ml_dtypes.bfloat16)
    ones[:, :, 0] = 1.0
    in_maps = []
    for c in range(NCORES):
        hs = slice(c * DH, (c + 1) * DH)
        wo_a = np.zeros((DH + 2, D + 2), dtype=ml_dtypes.bfloat16)
        wo_a[0:DH, 0:D] = Wo[hs, :]
        wo_a[DH, D] = 1.0
        in_maps.append({
            "xT": xT,
            "wq": np.ascontiguousarray(Wq[:, hs]).astype(ml_dtypes.bfloat16),
            "wk": np.ascontiguousarray(Wk[:, hs]).astype(ml_dtypes.bfloat16),
            "wv": np.ascontiguousarray(Wv[:, hs]).astype(ml_dtypes.bfloat16),
            "wo_aug": wo_a,
            "bq": np.ascontiguousarray(bq[hs].reshape(DH, 1)),
            "bk": np.ascontiguousarray(bk[hs].reshape(DH, 1)),
            "idin": eye,
            "onesin": ones,
        })

    res = run_bass_kernel_spmd(nc, in_maps, list(range(NCORES)))

    acc = np.zeros((B * S, D), dtype=np.float32)
    for c in range(NCORES):
        acc += res.results[c]["out"]
    # biases that commute with the head-reduction, applied at gather time
    acc += bo[None, :] + (bv @ Wo)[None, :]
    return acc.reshape(B, S, D)


# revision 24
# speedup vs baseline: 2.1441x; 1.0615x over previous
"""Trainium2 Bass kernel for nn_MultiHeadAttention (B=4, S=2048, D=512, H=8).

Sharding: tensor-parallel over heads — core c owns head c (Dh=64).
Each core computes q/k/v projections for its head slice (full x replicated,
host-pre-transposed to x^T in bf16), attention for its head over all 4
batches, and the partial out-projection O_c @ Wo[c]; the host sums the 8
partials (the TP all-reduce done at gather time) and adds the biases that
commute with that reduction (bo, bv@Wo).

Engine plan (engines execute their programs in order, so emission order IS
the software pipeline):
  - PE: projections (W-stationary bf16), V^T->V PE-transposes, row-tiled
    S^T pairs (dh=64 contraction on 64x128 array halves), AV with a ones
    column (softmax denominators ride in O^T_aug row 64), out-projection.
  - ACT: exclusively exp(S/8) — it is the pacing engine (~143us floor);
    anything else queued on ACT delays the next attention phase.
  - DVE: all PSUM evacuations + the final normalize (reciprocal of the
    denominator extracted per token partition by an e-column matmul).
  - GPSIMD: bulk x^T loads;  SP: staging shifts + output stores.
Batches are paired [even; odd] on SBUF partition halves; prep of pair 1 is
emitted interleaved with attention of pair 0, and both out-projections are
emitted interleaved with attention of pair 1.
"""
import numpy as np

import concourse.bass as bass
import concourse.mybir as mybir
import concourse.tile as tile
from concourse import bacc
from concourse.bass_utils import run_bass_kernel_spmd

B, S, D = 4, 2048, 512
H, DH = 8, 64
NCORES = 8
F32 = mybir.dt.float32
F32R = mybir.dt.float32r
BF16 = mybir.dt.bfloat16
AF = mybir.ActivationFunctionType

NKT = S // 128          # 16 token tiles per batch
NQB = S // 512          # 4 512-blocks per batch
NCH = D // 128          # 4 dm chunks

_NC_CACHE = {}


def build_kernel():
    nc = bacc.Bacc("TRN2", target_bir_lowering=False, debug=False)

    xT = nc.dram_tensor("xT", [B, D, S], BF16, kind="ExternalInput")
    wq = nc.dram_tensor("wq", [D, DH], BF16, kind="ExternalInput")
    wk = nc.dram_tensor("wk", [D, DH], BF16, kind="ExternalInput")
    wv = nc.dram_tensor("wv", [D, DH], BF16, kind="ExternalInput")
    wo_aug = nc.dram_tensor("wo_aug", [DH + 2, D + 2], BF16, kind="ExternalInput")
    bq = nc.dram_tensor("bq", [DH, 1], F32, kind="ExternalInput")
    bk = nc.dram_tensor("bk", [DH, 1], F32, kind="ExternalInput")
    idin = nc.dram_tensor("idin", [128, 128], BF16, kind="ExternalInput")
    onesin = nc.dram_tensor("onesin", [128, 16, 2], BF16, kind="ExternalInput")
    out = nc.dram_tensor("out", [B * S, D], F32, kind="ExternalOutput")

    with tile.TileContext(nc) as tc:
        with (
            tc.tile_pool(name="consts", bufs=1) as consts,
            tc.tile_pool(name="xtp", bufs=16) as xtp,
            tc.tile_pool(name="qkp", bufs=2) as qkp,
            tc.tile_pool(name="stgp", bufs=4) as stgp,
            tc.tile_pool(name="vtp", bufs=4) as vtp,
            tc.tile_pool(name="vp", bufs=4) as vp,
            tc.tile_pool(name="ptp", bufs=3) as ptp,
            tc.tile_pool(name="otp", bufs=4) as otp,
            tc.tile_pool(name="outp", bufs=6) as outp,
            tc.tile_pool(name="rcp", bufs=6) as rcp,
            tc.tile_pool(name="psA", bufs=2, space="PSUM") as psA,   # pst [128,1024] x2
            tc.tile_pool(name="psO", bufs=2, space="PSUM") as psO,   # po [66,512] x2
            tc.tile_pool(name="psM", bufs=2, space="PSUM") as psM,   # misc [128,512] x2
        ):
            wq_sb = consts.tile([128, NCH, DH], BF16)
            wk_sb = consts.tile([128, NCH, DH], BF16)
            wv_sb = consts.tile([128, NCH, DH], BF16)
            wo_sb = consts.tile([DH + 2, D + 2], BF16)
            bq_sb = consts.tile([DH, 1], F32)
            bk_sb = consts.tile([DH, 1], F32)
            ident = consts.tile([128, 128], BF16)
            nc.sync.dma_start(out=wq_sb[:], in_=wq.rearrange("(c p) m -> p c m", p=128))
            nc.sync.dma_start(out=wk_sb[:], in_=wk.rearrange("(c p) m -> p c m", p=128))
            nc.sync.dma_start(out=wv_sb[:], in_=wv.rearrange("(c p) m -> p c m", p=128))
            nc.sync.dma_start(out=wo_sb[:], in_=wo_aug[:])
            nc.sync.dma_start(out=bq_sb[:], in_=bq[:])
            nc.sync.dma_start(out=bk_sb[:], in_=bk[:])
            nc.sync.dma_start(out=ident[:], in_=idin[:])

            state = {}

            def alloc_pair(pr):
                st = {"xt": {}, "vt": {}, "v": {}, "ot": {}}
                st["qt"] = qkp.tile([128, S], BF16, tag="qt", name=f"qt_{pr}")
                st["kt"] = qkp.tile([128, S], BF16, tag="kt", name=f"kt_{pr}")
                for half in range(2):
                    b = pr * 2 + half
                    st["vt"][half] = vtp.tile([DH, S], BF16, tag="vt", name=f"vt_{b}")
                state[pr] = st

            def emit_xt_loads(pr):
                st = state[pr]
                for half in range(2):
                    b = pr * 2 + half
                    xts = []
                    for ci in range(NCH):
                        xt_c = xtp.tile([128, S], BF16, tag="xt", name=f"xt_{b}_{ci}")
                        xts.append(xt_c)
                    for blk in range(NQB):
                        for ci in range(NCH):
                            nc.gpsimd.dma_start(
                                out=xts[ci][:, bass.ts(blk, 512)],
                                in_=xT[b, bass.ts(ci, 128), bass.ts(blk, 512)],
                            )
                    st["xt"][half] = xts

            def emit_prep_qk(pr, blk, half):
                st = state[pr]
                sl = bass.ts(blk, 512)
                b = pr * 2 + half
                xt = st["xt"][half]
                pqk = psM.tile([128, 512], F32, tag="psM", name=f"pqk_{b}_{blk}")
                pq = pqk[0:DH, :]
                pk = pqk[DH:128, :]
                for ci in range(NCH):
                    nc.tensor.matmul(
                        pq, wq_sb[:, ci, :], xt[ci][:, sl],
                        start=(ci == 0), stop=(ci == NCH - 1),
                    )
                for ci in range(NCH):
                    nc.tensor.matmul(
                        pk, wk_sb[:, ci, :], xt[ci][:, sl],
                        start=(ci == 0), stop=(ci == NCH - 1),
                    )
                if half == 0:
                    nc.vector.tensor_scalar_add(st["qt"][0:DH, sl], pq, bq_sb[:])
                    nc.vector.tensor_scalar_add(st["kt"][0:DH, sl], pk, bk_sb[:])
                else:
                    sq = stgp.tile([DH, 512], BF16, tag="stg", name=f"sq_{b}_{blk}")
                    sk = stgp.tile([DH, 512], BF16, tag="stg", name=f"sk_{b}_{blk}")
                    nc.vector.tensor_scalar_add(sq[:], pq, bq_sb[:])
                    nc.vector.tensor_scalar_add(sk[:], pk, bk_sb[:])
                    nc.sync.dma_start(out=st["qt"][DH:128, sl], in_=sq[:])
                    nc.sync.dma_start(out=st["kt"][DH:128, sl], in_=sk[:])

            def emit_prep_v(pr, blk, half):
                st = state[pr]
                sl = bass.ts(blk, 512)
                b = pr * 2 + half
                xt = st["xt"][half]
                pvt = psM.tile([DH, 512], F32, tag="psM", name=f"pvt_{b}_{blk}")
                for ci in range(NCH):
                    nc.tensor.matmul(
                        pvt[:], wv_sb[:, ci, :], xt[ci][:, sl],
                        start=(ci == 0), stop=(ci == NCH - 1),
                    )
                nc.vector.tensor_copy(st["vt"][half][:, sl], pvt[:])

            def emit_vtr(pr, vh, half):
                st = state[pr]
                b = pr * 2 + half
                if vh == 0:
                    v_b = vp.tile([128, NKT, DH + 2], BF16, tag="v", name=f"v_{b}")
                    nc.gpsimd.dma_start(out=v_b[:, :, DH:DH + 2], in_=onesin[:])
                    st["v"][half] = v_b
                v_b = st["v"][half]
                pvtr = psM.tile([128, 512], BF16, tag="psM", name=f"pvtr_{b}_{vh}")
                for j in range(8):
                    nc.tensor.transpose(
                        pvtr[:, bass.ts(j, 64)],
                        st["vt"][half][:, bass.ts(vh * 8 + j, 128)],
                        ident[0:DH, 0:DH],
                    )
                nc.vector.tensor_copy(
                    v_b[:, bass.ds(vh * 8, 8), 0:DH],
                    pvtr[:].rearrange("p (k m) -> p k m", m=64),
                )

            def emit_attn_qq(pr, qq, fillers=None):
                st = state[pr]
                with nc.named_scope(f"attn_{pr}_{qq}"):
                    sl_q = bass.ts(qq, 512)
                    if qq == 0:
                        for half in range(2):
                            st["ot"][half] = otp.tile(
                                [DH + 2, S], BF16, tag="ot", name=f"ot_{pr * 2 + half}"
                            )
                    po = [
                        psO.tile([DH + 2, 512], F32, tag="psO", name=f"po{hb}_{pr}_{qq}")
                        for hb in range(2)
                    ]
                    for kt_i in range(NKT):
                        kt_sl = bass.ts(kt_i, 128)
                        pst = psA.tile([128, 1024], F32, tag="psA", name=f"pst_{pr}_{qq}_{kt_i}")
                        for hb in range(2):
                            nc.tensor.matmul(
                                pst[:, bass.ts(hb, 512)],
                                st["kt"][hb * DH:(hb + 1) * DH, kt_sl],
                                st["qt"][hb * DH:(hb + 1) * DH, sl_q],
                                start=True, stop=True,
                                tile_position=(hb * DH, 0),
                            )
                        ptt = ptp.tile([128, 1024], BF16, tag="pt", name=f"ptt_{pr}_{qq}_{kt_i}")
                        nc.scalar.activation(ptt[:], pst[:], AF.Exp, scale=0.125)
                        for hb in range(2):
                            nc.tensor.matmul(
                                po[hb][:],
                                st["v"][hb][:, kt_i, :],
                                ptt[:, bass.ts(hb, 512)],
                                start=(kt_i == 0), stop=(kt_i == NKT - 1),
                            )
                        if fillers and kt_i % 2 == 1 and fillers:
                            f = fillers.pop(0) if fillers else None
                            if f is not None:
                                f()
                    for hb in range(2):
                        nc.vector.tensor_copy(st["ot"][hb][:, sl_q], po[hb][:])

            def emit_op_tt(pr, half, tt):
                st = state[pr]
                b = pr * 2 + half
                ot_b = st["ot"][half]
                pop = psM.tile([128, 512], F32, tag="psM", name=f"pop_{b}_{tt}")
                pos = psM.tile([128, 2], F32, tag="psM", name=f"pos_{b}_{tt}")
                otc = ot_b[:, bass.ts(tt, 128)]
                nc.tensor.matmul(pop[:], otc, wo_sb[:, 0:D], start=True, stop=True)
                nc.tensor.matmul(pos[:], otc, wo_sb[:, D:D + 2], start=True, stop=True)
                rc = rcp.tile([128, 1], F32, tag="rc", name=f"rc_{b}_{tt}")
                nc.vector.reciprocal(rc[:], pos[:, 0:1])
                so = outp.tile([128, 512], F32, tag="so", name=f"so_{b}_{tt}")
                nc.vector.tensor_scalar_mul(so[:], pop[:], rc[:])
                nc.sync.dma_start(
                    out=out[bass.ds(b * S + tt * 128, 128), :], in_=so[:]
                )

            # ---------------- emission schedule ----------------
            import functools
            alloc_pair(0)
            alloc_pair(1)
            emit_xt_loads(0)
            emit_xt_loads(1)
            # pair-0 prep head (full)
            for blk in range(NQB):
                for half in range(2):
                    emit_prep_qk(0, blk, half)
                    emit_prep_v(0, blk, half)
                if blk in (1, 3):
                    for half in range(2):
                        emit_vtr(0, blk // 2, half)

            P = functools.partial
            fill0 = []   # consumed during attn(0, *): all of prep1
            for blk in range(NQB):
                for half in range(2):
                    fill0.append(P(emit_prep_qk, 1, blk, half))
                    fill0.append(P(emit_prep_v, 1, blk, half))
                if blk in (1, 3):
                    for half in range(2):
                        fill0.append(P(emit_vtr, 1, blk // 2, half))

            for qq in range(NQB):
                emit_attn_qq(0, qq, fillers=fill0)
            while fill0:
                fill0.pop(0)()

            fill1 = []   # consumed during attn(1, *): both out-projections
            for half in range(2):
                for tt in range(NKT):
                    fill1.append(P(emit_op_tt, 0, half, tt))
            # pair-1 OP pieces interleave after their q block is evacuated:
            # qq0 slots get op0; op1(qq) emitted during attn(1, qq+1)
            for qq in range(NQB):
                emit_attn_qq(1, qq, fillers=fill1)
                if qq >= 1:
                    for half in range(2):
                        for tt in range((qq - 1) * 4, qq * 4):
                            fill1.append(P(emit_op_tt, 1, half, tt))
            while fill1:
                fill1.pop(0)()
            for half in range(2):
                for tt in range(12, 16):
                    emit_op_tt(1, half, tt)

    nc.compile()
    return nc


def kernel(x, Wq, bq, Wk, bk, Wv, bv, Wo, bo):
    import ml_dtypes
    x = np.asarray(x, dtype=np.float32)
    xT = np.ascontiguousarray(np.transpose(x, (0, 2, 1))).astype(ml_dtypes.bfloat16)
    Wq = np.asarray(Wq, dtype=np.float32)
    Wk = np.asarray(Wk, dtype=np.float32)
    Wv = np.asarray(Wv, dtype=np.float32)
    Wo = np.asarray(Wo, dtype=np.float32)
    bq = np.asarray(bq, dtype=np.float32)
    bk = np.asarray(bk, dtype=np.float32)
    bv = np.asarray(bv, dtype=np.float32)
    bo = np.asarray(bo, dtype=np.float32)

    if "nc" not in _NC_CACHE:
        _NC_CACHE["nc"] = build_kernel()
    nc = _NC_CACHE["nc"]

    eye = np.eye(128).astype(ml_dtypes.bfloat16)
    ones = np.zeros((128, 16, 2), dtype=ml_dtypes.bfloat16)
    ones[:, :, 0] = 1.0
    in_maps = []
    for c in range(NCORES):
        hs = slice(c * DH, (c + 1) * DH)
        wo_a = np.zeros((DH + 2, D + 2), dtype=ml_dtypes.bfloat16)
        wo_a[0:DH, 0:D] = Wo[hs, :]
        wo_a[DH, D] = 1.0
        in_maps.append({
            "xT": xT,
            "wq": np.ascontiguousarray(Wq[:, hs]).astype(ml_dtypes.bfloat16),
            "wk": np.ascontiguousarray(Wk[:, hs]).astype(ml_dtypes.bfloat16),
            "wv": np.ascontiguousarray(Wv[:, hs]).astype(ml_dtypes.bfloat16),
            "wo_aug": wo_a,
            "bq": np.ascontiguousarray(bq[hs].reshape(DH, 1)),
            "bk": np.ascontiguousarray(bk[hs].reshape(DH, 1)),
            "idin": eye,
            "onesin": ones,
        })

    res = run_bass_kernel_spmd(nc, in_maps, list(range(NCORES)))

    acc = np.zeros((B * S, D), dtype=np.float32)
    for c in range(NCORES):
        acc += res.results[c]["out"]
    # biases that commute with the head-reduction, applied at gather time
    acc += bo[None, :] + (bv @ Wo)[None, :]
    return acc.reshape(B, S, D)
